# revision 1
# baseline (speedup 1.0000x reference)
"""Trainium2 Bass kernel: Conv3d(3->16, k=3, valid) + bias, min over D, softmax over C.

Full inputs: x [16,3,64,64,64], weight [16,3,3,3,3], bias [16].
Output: [16,16,62,62] f32.

Sharding: data-parallel, 2 samples per core across 8 cores.

Per-core algorithm (banded-weights matmul):
  - Contraction axis q = (h-window of 10 rows, kw of 3) = 30 partitions.
  - Stationary lhsT[q, m] with m = (h'-in-block of 8, co of 16) = 128:
      L[(hr,kw),(hp,co)] = W[co,ci,kd,hr-hp,kw] (banded, 0<=hr-hp<3),
      one per (ci,kd) -> 9 stationaries, accumulated in PSUM.
  - rhs = B tiles: partition (h,kw) = x[ci,:,h,kw:kw+62], free (d, w').
    kd enters as a free-dim offset (d'+kd), so only kw is replicated in SBUF.
  - min over d' via tensor_reduce(min) on PSUM chunks.
  - softmax over co (16-partition groups): exp via ACT (+bias fused),
    group-sum + group-broadcast via tiny matmuls with 0/1 selector matrices,
    multiply by reciprocal. Max-subtraction skipped: conv outputs are
    O(1)-scale normals, exp is well-conditioned.
"""

import sys

for _p in ("/opt/trn_rl_repo",):
    if _p not in sys.path:
        sys.path.insert(0, _p)

import numpy as np

import concourse.bass as bass
import concourse.tile as tile
from concourse import bacc, mybir
from concourse.bass_utils import run_bass_kernel_spmd

NS, CIN, CO = 2, 3, 16  # samples per core, in/out channels
D = H = W = 64
DO = HO = WO = 62
HALVES = [(0, 34), (32, 32)]  # (h_base, rows)
STARTS = [0, 8, 16, 24, 32, 40, 48, 54]  # h' block starts (last overlaps)

LAST_EXEC_NS = None

_nc_cache = None


def _build_nc():
    f32 = mybir.dt.float32
    nc = bacc.Bacc(None, target_bir_lowering=False)
    x = nc.dram_tensor("x", [NS, CIN, D, H, W], f32, kind="ExternalInput")
    lw = nc.dram_tensor("lw", [30, 9, 128], f32, kind="ExternalInput")
    aux = nc.dram_tensor("aux", [128, 9], f32, kind="ExternalInput")
    s16t = nc.dram_tensor("s16t", [8, 128], f32, kind="ExternalInput")
    y = nc.dram_tensor("y", [NS, CO, HO, WO], f32, kind="ExternalOutput")

    with tile.TileContext(nc) as tc:
        with (
            tc.tile_pool(name="wpool", bufs=1) as wpool,
            tc.tile_pool(name="bpool", bufs=3) as bpool,
            tc.tile_pool(name="work", bufs=3) as work,
            tc.tile_pool(name="cpsum", bufs=3, space="PSUM") as cpsum,
            tc.tile_pool(name="spsum", bufs=2, space="PSUM") as spsum,
        ):
            L = wpool.tile([30, 9, 128], f32)
            nc.sync.dma_start(out=L[:, :, :], in_=lw[:, :, :])
            A = wpool.tile([128, 9], f32)
            nc.sync.dma_start(out=A[:, :], in_=aux[:, :])
            S16T = wpool.tile([8, 128], f32)
            nc.sync.dma_start(out=S16T[:, :], in_=s16t[:, :])

            for n in range(NS):
                for bi, h0 in enumerate(STARTS):
                    # B tiles: partition (kw of 3, h_rel of 10), free (d, w')
                    bt = []
                    for ci in range(CIN):
                        t = bpool.tile([30, D, WO], f32, tag=f"b{ci}")
                        off = n * (CIN * D * H * W) + ci * (D * H * W) + h0 * W
                        for kw in range(3):
                            nc.sync.dma_start(
                                out=t[kw * 10 : kw * 10 + 10, :, :],
                                in_=bass.AP(
                                    x, off + kw, [[W, 10], [H * W, D], [1, WO]]
                                ),
                            )
                        bt.append(t)
                    acc = work.tile([128, WO], f32, tag="acc")
                    for c in range(8):
                        cs = 8 if c < 7 else 6
                        ps = cpsum.tile([128, 8, WO], f32, tag="cp")
                        r = 0
                        for ci in range(CIN):
                            for kd in range(3):
                                nc.tensor.matmul(
                                    ps[:, :cs, :],
                                    L[:, ci * 3 + kd, :],
                                    bt[ci][:, 8 * c + kd : 8 * c + kd + cs, :],
                                    start=(r == 0),
                                    stop=(r == 8),
                                )
                                r += 1
                        # min over the d' slots of this chunk
                        red_in = ps[:, :cs, :].rearrange("p d w -> p w d")
                        if c == 0:
                            nc.vector.tensor_reduce(
                                out=acc[:, :],
                                in_=red_in,
                                axis=mybir.AxisListType.X,
                                op=mybir.AluOpType.min,
                            )
                        else:
                            tmp = work.tile([128, WO], f32, tag="tmp")
                            nc.vector.tensor_reduce(
                                out=tmp[:, :],
                                in_=red_in,
                                axis=mybir.AxisListType.X,
                                op=mybir.AluOpType.min,
                            )
                            nc.vector.tensor_tensor(
                                out=acc[:, :],
                                in0=acc[:, :],
                                in1=tmp[:, :],
                                op=mybir.AluOpType.min,
                            )
                    # softmax over co within each 16-partition group
                    e = work.tile([128, WO], f32, tag="e")
                    nc.scalar.activation(
                        out=e[:, :],
                        in_=acc[:, :],
                        func=mybir.ActivationFunctionType.Exp,
                        bias=A[:, 0:1],
                    )
                    ps_s = spsum.tile([8, WO], f32, tag="ss")
                    nc.tensor.matmul(ps_s[:, :], A[:, 1:9], e[:, :], start=True, stop=True)
                    rs = work.tile([8, WO], f32, tag="rs")
                    nc.vector.reciprocal(out=rs[:, :], in_=ps_s[:, :])
                    ps_b = spsum.tile([128, WO], f32, tag="sb")
                    nc.tensor.matmul(ps_b[:, :], S16T[:, :], rs[:, :], start=True, stop=True)
                    o = work.tile([128, WO], f32, tag="o")
                    nc.vector.tensor_mul(o[:, :], e[:, :], ps_b[:, :])
                    ybase = n * (CO * HO * WO)
                    if bi < 7:
                        nc.sync.dma_start(
                            out=bass.AP(
                                y, ybase + h0 * WO, [[WO, 8], [HO * WO, CO], [1, WO]]
                            ),
                            in_=o[:, :],
                        )
                    else:
                        nc.sync.dma_start(
                            out=bass.AP(
                                y, ybase + 56 * WO, [[WO, 6], [HO * WO, CO], [1, WO]]
                            ),
                            in_=o[32:, :],
                        )
    nc.finalize()
    return nc


def _host_consts(weight, bias):
    lw = np.zeros((30, 9, 128), np.float32)
    for ci in range(CIN):
        for kd in range(3):
            for hr in range(10):
                for kw in range(3):
                    for hp in range(8):
                        kh = hr - hp
                        if 0 <= kh < 3:
                            lw[kw * 10 + hr, ci * 3 + kd, hp * 16 : hp * 16 + 16] = (
                                weight[:, ci, kd, kh, kw]
                            )
    aux = np.zeros((128, 9), np.float32)
    aux[:, 0] = np.tile(bias.astype(np.float32), 8)
    for p in range(128):
        aux[p, 1 + p // 16] = 1.0
    s16t = np.zeros((8, 128), np.float32)
    for p in range(128):
        s16t[p // 16, p] = 1.0
    return lw, aux, s16t


def kernel(x, weight, bias, _trace=False):
    global LAST_EXEC_NS, _nc_cache
    x = np.ascontiguousarray(x, dtype=np.float32)
    lw, aux, s16t = _host_consts(
        np.asarray(weight, np.float32), np.asarray(bias, np.float32)
    )
    if _nc_cache is None:
        _nc_cache = _build_nc()
    n_cores = 8
    in_maps = [
        {"x": np.ascontiguousarray(x[2 * k : 2 * k + 2]), "lw": lw, "aux": aux, "s16t": s16t}
        for k in range(n_cores)
    ]
    res = run_bass_kernel_spmd(_nc_cache, in_maps, list(range(n_cores)), trace=_trace)
    LAST_EXEC_NS = res.exec_time_ns
    out = np.concatenate([res.results[k]["y"] for k in range(n_cores)], axis=0)
    return out.astype(np.float32)


if __name__ == "__main__":
    rng = np.random.default_rng(0)
    x = rng.standard_normal((16, 3, 64, 64, 64), dtype=np.float32)
    w = rng.standard_normal((16, 3, 3, 3, 3), dtype=np.float32) / 9.0
    b = (rng.standard_normal(16) * 0.01).astype(np.float32)
    out = kernel(x, w, b)
    print("out", out.shape, out.dtype, out[0, :, 0, 0])



# revision 3
# speedup vs baseline: 2.9818x; 2.9818x over previous
"""Trainium2 Bass kernel: Conv3d(3->16, k=3, valid) + bias, min over D, softmax over C.

Full inputs: x [16,3,64,64,64], weight [16,3,3,3,3], bias [16].
Output: [16,16,62,62] f32.

Sharding: data-parallel, 2 samples per core across 8 cores.

Per-core algorithm (banded-weights matmul, v2):
  - Contraction axis q = (ci of 3, kw of 3, h-window of 10) = 90 partitions.
  - Stationary lhsT[q, m] with m = (h'-in-block of 8, co of 16) = 128:
      L[(ci,kw,hr), kd, (hp,co)] = W[co,ci,kd,hr-hp,kw] (banded, 0<=hr-hp<3),
      one per kd -> 3 accumulation steps in PSUM (vs 9 in v1).
  - Matmuls run as float32r (TF32-like): 1 cycle/row at free dim >= 256,
    4x faster than plain fp32.
  - rhs = one 90-partition tile per (n, h-block): partition (ci,kw,hr) holds
    x[ci, :, h0+hr, kw:kw+62]; kd enters as a free-dim offset (d'+kd).
  - min over d' via tensor_reduce(min) on PSUM chunks of 8 d' x 62 w.
  - softmax over co (16-partition groups): exp via ACT (+bias fused),
    group-sum + group-broadcast via tiny matmuls with 0/1 selector matrices,
    multiply by reciprocal.
"""

import sys

for _p in ("/opt/trn_rl_repo",):
    if _p not in sys.path:
        sys.path.insert(0, _p)

import numpy as np

import concourse.bass as bass
import concourse.tile as tile
from concourse import bacc, mybir
from concourse.bass_utils import run_bass_kernel_spmd

NS, CIN, CO = 2, 3, 16  # samples per core, in/out channels
D = H = W = 64
DO = HO = WO = 62
STARTS = [0, 8, 16, 24, 32, 40, 48, 54]  # h' block starts (last overlaps)

LAST_EXEC_NS = None

_nc_cache = None


def _build_nc():
    f32 = mybir.dt.float32
    f32r = mybir.dt.float32r
    nc = bacc.Bacc(None, target_bir_lowering=False)
    x = nc.dram_tensor("x", [NS, CIN, D, H, W], f32r, kind="ExternalInput")
    lw = nc.dram_tensor("lw", [90, 3, 128], f32r, kind="ExternalInput")
    aux = nc.dram_tensor("aux", [128, 9], f32, kind="ExternalInput")
    s16t = nc.dram_tensor("s16t", [8, 128], f32, kind="ExternalInput")
    y = nc.dram_tensor("y", [NS, CO, HO, WO], f32, kind="ExternalOutput")

    with tile.TileContext(nc) as tc:
        with (
            tc.tile_pool(name="wpool", bufs=1) as wpool,
            tc.tile_pool(name="bpool", bufs=3) as bpool,
            tc.tile_pool(name="work", bufs=3) as work,
            tc.tile_pool(name="cpsum", bufs=4, space="PSUM") as cpsum,
            tc.tile_pool(name="spsum", bufs=2, space="PSUM") as spsum,
        ):
            L = wpool.tile([90, 3, 128], f32r)
            nc.sync.dma_start(out=L[:, :, :], in_=lw[:, :, :])
            A = wpool.tile([128, 9], f32)
            nc.sync.dma_start(out=A[:, :], in_=aux[:, :])
            S16T = wpool.tile([8, 128], f32)
            nc.sync.dma_start(out=S16T[:, :], in_=s16t[:, :])

            for n in range(NS):
                for bi, h0 in enumerate(STARTS):
                    # rhs tile: partition (ci of 3, kw of 3, h_rel of 10),
                    # free (d, w')
                    bt = bpool.tile([90, D, WO], f32r, tag="b")
                    for ci in range(CIN):
                        off = n * (CIN * D * H * W) + ci * (D * H * W) + h0 * W
                        for kw in range(3):
                            nc.sync.dma_start(
                                out=bt[ci * 30 + kw * 10 : ci * 30 + kw * 10 + 10, :, :],
                                in_=bass.AP(
                                    x, off + kw, [[W, 10], [H * W, D], [1, WO]]
                                ),
                            )
                    acc = work.tile([128, WO], f32, tag="acc")
                    for c in range(8):
                        cs = 8 if c < 7 else 6
                        ps = cpsum.tile([128, 8, WO], f32, tag="cp")
                        for kd in range(3):
                            nc.tensor.matmul(
                                ps[:, :cs, :],
                                L[:, kd, :],
                                bt[:, 8 * c + kd : 8 * c + kd + cs, :],
                                start=(kd == 0),
                                stop=(kd == 2),
                            )
                        # min over the d' slots of this chunk
                        red_in = ps[:, :cs, :].rearrange("p d w -> p w d")
                        if c == 0:
                            nc.vector.tensor_reduce(
                                out=acc[:, :],
                                in_=red_in,
                                axis=mybir.AxisListType.X,
                                op=mybir.AluOpType.min,
                            )
                        else:
                            tmp = work.tile([128, WO], f32, tag="tmp")
                            nc.vector.tensor_reduce(
                                out=tmp[:, :],
                                in_=red_in,
                                axis=mybir.AxisListType.X,
                                op=mybir.AluOpType.min,
                            )
                            nc.vector.tensor_tensor(
                                out=acc[:, :],
                                in0=acc[:, :],
                                in1=tmp[:, :],
                                op=mybir.AluOpType.min,
                            )
                    # softmax over co within each 16-partition group
                    e = work.tile([128, WO], f32, tag="e")
                    nc.scalar.activation(
                        out=e[:, :],
                        in_=acc[:, :],
                        func=mybir.ActivationFunctionType.Exp,
                        bias=A[:, 0:1],
                    )
                    ps_s = spsum.tile([8, WO], f32, tag="ss")
                    nc.tensor.matmul(ps_s[:, :], A[:, 1:9], e[:, :], start=True, stop=True)
                    rs = work.tile([8, WO], f32, tag="rs")
                    nc.vector.reciprocal(out=rs[:, :], in_=ps_s[:, :])
                    ps_b = spsum.tile([128, WO], f32, tag="sb")
                    nc.tensor.matmul(ps_b[:, :], S16T[:, :], rs[:, :], start=True, stop=True)
                    o = work.tile([128, WO], f32, tag="o")
                    nc.vector.tensor_mul(o[:, :], e[:, :], ps_b[:, :])
                    ybase = n * (CO * HO * WO)
                    if bi < 7:
                        nc.sync.dma_start(
                            out=bass.AP(
                                y, ybase + h0 * WO, [[WO, 8], [HO * WO, CO], [1, WO]]
                            ),
                            in_=o[:, :],
                        )
                    else:
                        nc.sync.dma_start(
                            out=bass.AP(
                                y, ybase + 56 * WO, [[WO, 6], [HO * WO, CO], [1, WO]]
                            ),
                            in_=o[32:, :],
                        )
    nc.finalize()
    return nc


def _host_consts(weight, bias):
    # L[(ci,kw,hr), kd, (hp,co)] = w[co,ci,kd,hr-hp,kw] banded
    lw = np.zeros((90, 3, 128), np.float32)
    for ci in range(CIN):
        for kw in range(3):
            for hr in range(10):
                for kd in range(3):
                    for hp in range(8):
                        kh = hr - hp
                        if 0 <= kh < 3:
                            lw[ci * 30 + kw * 10 + hr, kd, hp * 16 : hp * 16 + 16] = (
                                weight[:, ci, kd, kh, kw]
                            )
    aux = np.zeros((128, 9), np.float32)
    aux[:, 0] = np.tile(bias.astype(np.float32), 8)
    for p in range(128):
        aux[p, 1 + p // 16] = 1.0
    s16t = np.zeros((8, 128), np.float32)
    for p in range(128):
        s16t[p // 16, p] = 1.0
    return lw, aux, s16t


def kernel(x, weight, bias, _trace=False):
    global LAST_EXEC_NS, _nc_cache
    x = np.ascontiguousarray(x, dtype=np.float32)
    lw, aux, s16t = _host_consts(
        np.asarray(weight, np.float32), np.asarray(bias, np.float32)
    )
    if _nc_cache is None:
        _nc_cache = _build_nc()
    n_cores = 8
    in_maps = [
        {"x": np.ascontiguousarray(x[2 * k : 2 * k + 2]), "lw": lw, "aux": aux, "s16t": s16t}
        for k in range(n_cores)
    ]
    res = run_bass_kernel_spmd(_nc_cache, in_maps, list(range(n_cores)), trace=_trace)
    LAST_EXEC_NS = res.exec_time_ns
    out = np.concatenate([res.results[k]["y"] for k in range(n_cores)], axis=0)
    return out.astype(np.float32)


if __name__ == "__main__":
    rng = np.random.default_rng(0)
    x = rng.standard_normal((16, 3, 64, 64, 64), dtype=np.float32)
    w = rng.standard_normal((16, 3, 3, 3, 3), dtype=np.float32) / 9.0
    b = (rng.standard_normal(16) * 0.01).astype(np.float32)
    out = kernel(x, w, b)
    print("out", out.shape, out.dtype, out[0, :, 0, 0])


# revision 4
# speedup vs baseline: 4.4999x; 1.5091x over previous
"""Trainium2 Bass kernel: Conv3d(3->16, k=3, valid) + bias, min over D, softmax over C.

Full inputs: x [16,3,64,64,64], weight [16,3,3,3,3], bias [16].
Output: [16,16,62,62] f32.

Sharding: data-parallel, 2 samples per core across 8 cores.

Per-core algorithm v3 (d-banded weights matmul, fat-descriptor DMA):
  - Band over D: out partitions m = (dp of 8, co of 16) = 128, contraction
    q = (ci of 3, kw of 3, d-window of 10) = 90, kh via 3 PSUM accumulation
    steps (free-dim row offset h'+kh). float32r matmuls: 1 cycle/row.
  - rhs tile per (n, d-block): partition (ci,kw,dr) holds the full 16KB
    contiguous (h,w)-plane x[n,ci,d0+dr,:,:], written at flat offset (2-kw)
    so the per-partition kw shift is baked into the data placement while the
    matmul reads a common free AP. DMA descriptors are 16KB each -> full
    DMA rate, cheap descriptor generation.
  - min over d' = tensor_tensor(min) merges of the 8 d-block PSUMs into
    acc[(dp,co), h'w'] (partition-aligned; overlap of the last block is an
    idempotent min). Remaining min over the 8 dp partition-groups: DMA
    shuffle acc -> acc2[(oct,co), (dp, 481)] then a single 128-lane
    tensor_reduce per sample.
  - softmax over co: exp via ACT (+bias), group-sum-broadcast via one
    128x128 0/1 selector matmul in the (oct,co) layout, reciprocal, mul.
"""

import sys

for _p in ("/opt/trn_rl_repo",):
    if _p not in sys.path:
        sys.path.insert(0, _p)

import numpy as np

import concourse.bass as bass
import concourse.tile as tile
from concourse import bacc, mybir
from concourse.bass_utils import run_bass_kernel_spmd

NS, CIN, CO = 2, 3, 16  # samples per core, in/out channels
D = H = W = 64
DO = HO = WO = 62
SP = HO * WO  # 3844 spatial outputs per (n, co)
OCT = 481  # ceil(3844/8); acc pad rows make reads of [3844,3848) safe
DSTARTS = [0, 8, 16, 24, 32, 40, 48, 54]  # d-block starts (last overlaps)
HCH = [(0, 8), (8, 8), (16, 8), (24, 8), (32, 8), (40, 8), (48, 8), (56, 6)]

LAST_EXEC_NS = None

_nc_cache = None


def _build_nc():
    f32 = mybir.dt.float32
    f32r = mybir.dt.float32r
    nc = bacc.Bacc(None, target_bir_lowering=False)
    x = nc.dram_tensor("x", [NS, CIN, D, H, W], f32r, kind="ExternalInput")
    lw = nc.dram_tensor("lw", [90, 3, 128], f32r, kind="ExternalInput")
    sel = nc.dram_tensor("sel", [128, 128], f32, kind="ExternalInput")
    bia = nc.dram_tensor("bia", [128, 1], f32, kind="ExternalInput")
    y = nc.dram_tensor("y", [NS, CO, HO, WO], f32, kind="ExternalOutput")

    CDHW = CIN * D * H * W
    DHW = D * H * W
    HW = H * W

    with tile.TileContext(nc) as tc:
        with (
            tc.tile_pool(name="wpool", bufs=1) as wpool,
            tc.tile_pool(name="bpool", bufs=3) as bpool,
            tc.tile_pool(name="apool", bufs=2) as apool,
            tc.tile_pool(name="work", bufs=2) as work,
            tc.tile_pool(name="cpsum", bufs=4, space="PSUM") as cpsum,
            tc.tile_pool(name="spsum", bufs=2, space="PSUM") as spsum,
        ):
            L = wpool.tile([90, 3, 128], f32r)
            nc.sync.dma_start(out=L[:, :, :], in_=lw[:, :, :])
            SEL = wpool.tile([128, 128], f32)
            nc.sync.dma_start(out=SEL[:, :], in_=sel[:, :])
            BIA = wpool.tile([128, 1], f32)
            nc.sync.dma_start(out=BIA[:, :], in_=bia[:, :])

            for n in range(NS):
                # acc[(dp,co), h', w'] with 2 pad rows (rows 62,63)
                acc = apool.tile([128, 64, WO], f32, tag="acc")
                for bi, d0 in enumerate(DSTARTS):
                    # rhs tile: partition (ci,kw,dr); free = padded flat plane
                    bt = bpool.tile([90, 65, 64], f32r, tag="b")
                    btf = bt.rearrange("p a b -> p (a b)")
                    for ci in range(CIN):
                        for kw in range(3):
                            p0 = ci * 30 + kw * 10
                            nc.sync.dma_start(
                                out=btf[p0 : p0 + 10, (2 - kw) : (2 - kw) + HW],
                                in_=bass.AP(
                                    x,
                                    n * CDHW + ci * DHW + d0 * HW,
                                    [[HW, 10], [1, HW]],
                                ),
                            )
                    for c, (h0, hs) in enumerate(HCH):
                        ps = cpsum.tile([128, 8, WO], f32, tag="cp")
                        for kh in range(3):
                            nc.tensor.matmul(
                                ps[:, :hs, :],
                                L[:, kh, :],
                                bt[:, h0 + kh : h0 + kh + hs, 2:64],
                                start=(kh == 0),
                                stop=(kh == 2),
                            )
                        if bi == 0:
                            nc.scalar.copy(
                                out=acc[:, h0 : h0 + hs, :], in_=ps[:, :hs, :]
                            )
                        else:
                            nc.vector.tensor_tensor(
                                out=acc[:, h0 : h0 + hs, :],
                                in0=acc[:, h0 : h0 + hs, :],
                                in1=ps[:, :hs, :],
                                op=mybir.AluOpType.min,
                            )
                # tail: shuffle to [(oct,co), (dp, 481)] and reduce over dp
                accf = acc.rearrange("p a b -> p (a b)")
                a2 = apool.tile([128, 8, OCT], f32, tag="a2")
                for oct in range(8):
                    for dp in range(8):
                        nc.scalar.dma_start(
                            out=a2[oct * 16 : oct * 16 + 16, dp, :],
                            in_=accf[dp * 16 : dp * 16 + 16, oct * OCT : (oct + 1) * OCT],
                        )
                m = work.tile([128, OCT], f32, tag="m")
                nc.vector.tensor_reduce(
                    out=m[:, :],
                    in_=a2.rearrange("p d j -> p j d"),
                    axis=mybir.AxisListType.X,
                    op=mybir.AluOpType.min,
                )
                e = work.tile([128, OCT], f32, tag="e")
                nc.scalar.activation(
                    out=e[:, :],
                    in_=m[:, :],
                    func=mybir.ActivationFunctionType.Exp,
                    bias=BIA[:, 0:1],
                )
                ss = spsum.tile([128, OCT], f32, tag="ss")
                nc.tensor.matmul(ss[:, :], SEL[:, :], e[:, :], start=True, stop=True)
                r = work.tile([128, OCT], f32, tag="r")
                nc.vector.reciprocal(out=r[:, :], in_=ss[:, :])
                o = work.tile([128, OCT], f32, tag="o")
                nc.vector.tensor_mul(o[:, :], e[:, :], r[:, :])
                ybase = n * (CO * SP)
                for oct in range(8):
                    w_ = OCT if oct < 7 else SP - 7 * OCT
                    nc.sync.dma_start(
                        out=bass.AP(y, ybase + oct * OCT, [[SP, 16], [1, w_]]),
                        in_=o[oct * 16 : oct * 16 + 16, :w_],
                    )
    nc.finalize()
    return nc


def _host_consts(weight, bias):
    # L[(ci,kw,dr), kh, (dp,co)] = w[co,ci,dr-dp,kh,kw] banded
    lw = np.zeros((90, 3, 128), np.float32)
    for ci in range(CIN):
        for kw in range(3):
            for dr in range(10):
                for kh in range(3):
                    for dp in range(8):
                        kd = dr - dp
                        if 0 <= kd < 3:
                            lw[ci * 30 + kw * 10 + dr, kh, dp * 16 : dp * 16 + 16] = (
                                weight[:, ci, kd, kh, kw]
                            )
    sel = np.zeros((128, 128), np.float32)
    for oct in range(8):
        sel[oct * 16 : oct * 16 + 16, oct * 16 : oct * 16 + 16] = 1.0
    bia = np.tile(bias.astype(np.float32), 8).reshape(128, 1)
    return lw, sel, bia


def kernel(x, weight, bias, _trace=False):
    global LAST_EXEC_NS, _nc_cache
    x = np.ascontiguousarray(x, dtype=np.float32)
    lw, sel, bia = _host_consts(
        np.asarray(weight, np.float32), np.asarray(bias, np.float32)
    )
    if _nc_cache is None:
        _nc_cache = _build_nc()
    n_cores = 8
    in_maps = [
        {"x": np.ascontiguousarray(x[2 * k : 2 * k + 2]), "lw": lw, "sel": sel, "bia": bia}
        for k in range(n_cores)
    ]
    res = run_bass_kernel_spmd(_nc_cache, in_maps, list(range(n_cores)), trace=_trace)
    LAST_EXEC_NS = res.exec_time_ns
    out = np.concatenate([res.results[k]["y"] for k in range(n_cores)], axis=0)
    return out.astype(np.float32)


if __name__ == "__main__":
    rng = np.random.default_rng(0)
    x = rng.standard_normal((16, 3, 64, 64, 64), dtype=np.float32)
    w = rng.standard_normal((16, 3, 3, 3, 3), dtype=np.float32) / 9.0
    b = (rng.standard_normal(16) * 0.01).astype(np.float32)
    out = kernel(x, w, b)
    print("out", out.shape, out.dtype, out[0, :, 0, 0])


# revision 8
# speedup vs baseline: 6.6205x; 1.4713x over previous
"""Trainium2 Bass kernel: Conv3d(3->16, k=3, valid) + bias, min over D, softmax over C.

Full inputs: x [16,3,64,64,64], weight [16,3,3,3,3], bias [16].
Output: [16,16,62,62] f32.

Sharding: data-parallel, 2 samples per core across 8 cores.

Per-core algorithm v4 (d-banded bf16 matmul, staged bf16 replication):
  - Band over D: out partitions m = (dp of 8, co of 16) = 128, contraction
    q = (kw of 3, dr of 10, ci of 3) = 90, kh via 3 PSUM accumulation steps
    (free-dim row offset h'+kh). bf16 matmuls: 1 cycle/row, FWL weight loads.
  - Staging: per sample one gpsimd SWDGE DMA loads x[n] f32 from HBM and
    casts inline to bf16 [64(d), ci, 4096] in SBUF (16KB descriptors).
  - rhs tile per (n, d-block): partition (kw,dr,ci) holds the bf16 (h,w)
    plane x[n,ci,d0+dr,:,:] written at flat element offset (2-kw), so the
    per-partition kw shift is baked into placement while the matmul reads a
    common free AP. Built by 3 SBUF->SBUF copies (one per kw, 30 contiguous
    partitions, 8KB descriptors), alternating sync/scalar DMA queues.
  - min over d' = tensor_tensor(min) merges of the 8 d-block PSUMs into
    acc[(dp,co), h', w'] (idempotent-overlap last block), then a 3-step
    gpsimd tree-min over the dp partition groups -> m at partitions 0..15.
  - softmax over co: ACT exp(+bias) -> bf16, all-ones 16x16 matmul for the
    channel sum (per 496-col chunk), DVE reciprocal + multiply, one fat
    output DMA per sample.
"""

import sys

for _p in ("/opt/trn_rl_repo",):
    if _p not in sys.path:
        sys.path.insert(0, _p)

import ml_dtypes
import numpy as np

import concourse.bass as bass
import concourse.tile as tile
from concourse import bacc, mybir
from concourse.bass_utils import run_bass_kernel_spmd

NS, CIN, CO = 2, 3, 16  # samples per core, in/out channels
D = H = W = 64
DO = HO = WO = 62
SP = HO * WO  # 3844 spatial outputs per (n, co)
DSTARTS = [0, 8, 16, 24, 32, 40, 48, 54]  # d-block starts (last overlaps)
HCH = [(0, 8), (8, 8), (16, 8), (24, 8), (32, 8), (40, 8), (48, 8), (56, 6)]

LAST_EXEC_NS = None

_nc_cache = None


def _build_nc():
    f32 = mybir.dt.float32
    bf16 = mybir.dt.bfloat16
    nc = bacc.Bacc(None, target_bir_lowering=False)
    x = nc.dram_tensor("x", [NS, CIN, D, H, W], f32, kind="ExternalInput")
    lw = nc.dram_tensor("lw", [90, 3, 128], bf16, kind="ExternalInput")
    sel = nc.dram_tensor("sel", [16, 16], bf16, kind="ExternalInput")
    bia = nc.dram_tensor("bia", [16, 1], f32, kind="ExternalInput")
    y = nc.dram_tensor("y", [NS, CO, HO, WO], f32, kind="ExternalOutput")

    CDHW = CIN * D * H * W
    DHW = D * H * W
    HW = H * W

    with tile.TileContext(nc) as tc:
        with (
            tc.tile_pool(name="wpool", bufs=1) as wpool,
            tc.tile_pool(name="stage", bufs=2) as stage,
            tc.tile_pool(name="bpool", bufs=4) as bpool,
            tc.tile_pool(name="apool", bufs=2) as apool,
            tc.tile_pool(name="work", bufs=2) as work,
            tc.tile_pool(name="cpsum", bufs=4, space="PSUM") as cpsum,
            tc.tile_pool(name="spsum", bufs=2, space="PSUM") as spsum,
        ):
            L = wpool.tile([90, 3, 128], bf16)
            nc.sync.dma_start(out=L[:, :, :], in_=lw[:, :, :])
            SEL = wpool.tile([16, 16], bf16)
            nc.sync.dma_start(out=SEL[:, :], in_=sel[:, :])
            BIA = wpool.tile([16, 1], f32)
            nc.sync.dma_start(out=BIA[:, :], in_=bia[:, :])

            qeng = [nc.sync, nc.scalar]
            for n in range(NS):
                # bf16 staged planes: [d, ci, 4096], cast inline from f32 HBM
                sg = stage.tile([D, CIN, HW], bf16, tag="sg")
                nc.gpsimd.dma_start(
                    out=sg[:, :, :],
                    in_=bass.AP(x, n * CDHW, [[HW, D], [DHW, CIN], [1, HW]]),
                )
                acc = apool.tile([128, 64, WO], f32, tag="acc")
                for bi, d0 in enumerate(DSTARTS):
                    # rhs tile: partition (kw, dr, ci); free = padded flat plane
                    bt = bpool.tile([90, 65, 64], bf16, tag="b")
                    btf = bt.rearrange("p a b -> p (a b)")
                    for kw in range(3):
                        qeng[(bi * 3 + kw) % 2].dma_start(
                            out=btf[kw * 30 : kw * 30 + 30, (2 - kw) : (2 - kw) + HW],
                            in_=sg[d0 : d0 + 10, :, :],
                        )
                    for c, (h0, hs) in enumerate(HCH):
                        ps = cpsum.tile([128, 8, WO], f32, tag="cp")
                        for kh in range(3):
                            nc.tensor.matmul(
                                ps[:, :hs, :],
                                L[:, kh, :],
                                bt[:, h0 + kh : h0 + kh + hs, 2:64],
                                start=(kh == 0),
                                stop=(kh == 2),
                            )
                        if bi == 0:
                            nc.scalar.copy(
                                out=acc[:, h0 : h0 + hs, :], in_=ps[:, :hs, :]
                            )
                        else:
                            nc.vector.tensor_tensor(
                                out=acc[:, h0 : h0 + hs, :],
                                in0=acc[:, h0 : h0 + hs, :],
                                in1=ps[:, :hs, :],
                                op=mybir.AluOpType.min,
                            )
                # tree-min over the 8 dp partition groups: ACT copies the
                # upper half down to base partition 0 (single-input ops may
                # cross base partitions), gpsimd does the aligned min.
                ts = work.tile([64, 64, WO], f32, tag="ts")
                for half in (64, 32, 16):
                    if half >= 32:
                        # ACT reads must start at a partition multiple of 32
                        nc.scalar.copy(
                            out=ts[0:half, :, :], in_=acc[half : 2 * half, :, :]
                        )
                    else:
                        nc.sync.dma_start(
                            out=ts[0:half, :, :], in_=acc[half : 2 * half, :, :]
                        )
                    nc.vector.tensor_tensor(
                        out=acc[0:half, :, :],
                        in0=acc[0:half, :, :],
                        in1=ts[0:half, :, :],
                        op=mybir.AluOpType.min,
                    )
                # softmax over co on partitions 0..15
                e = work.tile([16, 64, WO], bf16, tag="e")
                nc.scalar.activation(
                    out=e[:, :, :],
                    in_=acc[0:16, :, :],
                    func=mybir.ActivationFunctionType.Exp,
                    bias=BIA[:, 0:1],
                )
                o = work.tile([16, 64, WO], f32, tag="o")
                for c, (h0, hs) in enumerate(HCH):
                    ss = spsum.tile([16, 8, WO], f32, tag="ss")
                    nc.tensor.matmul(
                        ss[:, :hs, :],
                        SEL[:, :],
                        e[:, h0 : h0 + hs, :],
                        start=True,
                        stop=True,
                    )
                    r = work.tile([16, 8, WO], f32, tag="r")
                    nc.vector.reciprocal(out=r[:, :hs, :], in_=ss[:, :hs, :])
                    nc.vector.tensor_mul(
                        o[:, h0 : h0 + hs, :], e[:, h0 : h0 + hs, :], r[:, :hs, :]
                    )
                of = o.rearrange("p a b -> p (a b)")
                nc.sync.dma_start(
                    out=bass.AP(y, n * CO * SP, [[SP, 16], [1, SP]]),
                    in_=of[:, 0:SP],
                )
    nc.finalize()
    return nc


def _host_consts(weight, bias):
    # L[(kw,dr,ci), kh, (dp,co)] = w[co,ci,dr-dp,kh,kw] banded
    lw = np.zeros((90, 3, 128), np.float32)
    for kw in range(3):
        for dr in range(10):
            for ci in range(CIN):
                for kh in range(3):
                    for dp in range(8):
                        kd = dr - dp
                        if 0 <= kd < 3:
                            lw[kw * 30 + dr * 3 + ci, kh, dp * 16 : dp * 16 + 16] = (
                                weight[:, ci, kd, kh, kw]
                            )
    lw = lw.astype(ml_dtypes.bfloat16)
    sel = np.ones((16, 16), ml_dtypes.bfloat16)
    bia = bias.astype(np.float32).reshape(16, 1)
    return lw, sel, bia


def kernel(x, weight, bias, _trace=False):
    global LAST_EXEC_NS, _nc_cache
    x = np.ascontiguousarray(x, dtype=np.float32)
    lw, sel, bia = _host_consts(
        np.asarray(weight, np.float32), np.asarray(bias, np.float32)
    )
    if _nc_cache is None:
        _nc_cache = _build_nc()
    n_cores = 8
    in_maps = [
        {"x": np.ascontiguousarray(x[2 * k : 2 * k + 2]), "lw": lw, "sel": sel, "bia": bia}
        for k in range(n_cores)
    ]
    res = run_bass_kernel_spmd(_nc_cache, in_maps, list(range(n_cores)), trace=_trace)
    LAST_EXEC_NS = res.exec_time_ns
    out = np.concatenate([res.results[k]["y"] for k in range(n_cores)], axis=0)
    return out.astype(np.float32)


if __name__ == "__main__":
    rng = np.random.default_rng(0)
    x = rng.standard_normal((16, 3, 64, 64, 64), dtype=np.float32)
    w = rng.standard_normal((16, 3, 3, 3, 3), dtype=np.float32) / 9.0
    b = (rng.standard_normal(16) * 0.01).astype(np.float32)
    out = kernel(x, w, b)
    print("out", out.shape, out.dtype, out[0, :, 0, 0])


# revision 9
# speedup vs baseline: 8.3233x; 1.2572x over previous
"""Trainium2 Bass kernel: Conv3d(3->16, k=3, valid) + bias, min over D, softmax over C.

Full inputs: x [16,3,64,64,64], weight [16,3,3,3,3], bias [16].
Output: [16,16,62,62] f32.

Sharding: data-parallel, 2 samples per core across 8 cores.

Per-core algorithm v5 (d-banded bf16 matmul, direct cast loads):
  - Band over D: out partitions m = (dp of 8, co of 16) = 128, contraction
    q = (kw of 3, dr of 10, ci of 3) = 90, kh via 3 PSUM accumulation steps
    (free-dim row offset h'+kh). bf16 matmuls: 1 cycle/row, FWL weight loads.
  - rhs tile per (n, d-block): partition (kw,dr,ci) holds the bf16 (h,w)
    plane x[n,ci,d0+dr,:,:] at flat offset (2-kw) (kw shift baked into
    placement). Loaded as: one gpsimd SWDGE cast-DMA (f32 HBM -> bf16, 16KB
    descriptors) into the kw=1 partitions, then two SBUF->SBUF bf16 copies
    into the kw=0/2 partitions on the sync/scalar HWDGE queues.
  - min over d' merges the 8 d-block PSUMs into bf16 acc[(dp,co), h', w'];
    chunks alternate between a direct DVE min (PSUM f32 operand) and an
    ACT copy->bf16 scratch + DVE 4x-mode bf16 min, balancing both engines.
  - dp-group tree-min: gpsimd-queue DMA copies the upper partition half to
    base 0, DVE bf16 min; 3 steps -> m at partitions 0..15.
  - softmax over co: ACT exp(+bias) -> bf16, all-ones 16x16 matmul for the
    channel sum per 496-col chunk, DVE reciprocal_approx_fast + multiply,
    one fat output DMA per sample.
"""

import sys

for _p in ("/opt/trn_rl_repo",):
    if _p not in sys.path:
        sys.path.insert(0, _p)

import ml_dtypes
import numpy as np

import concourse.bass as bass
import concourse.tile as tile
from concourse import bacc, mybir
from concourse.bass_utils import run_bass_kernel_spmd

NS, CIN, CO = 2, 3, 16  # samples per core, in/out channels
D = H = W = 64
DO = HO = WO = 62
SP = HO * WO  # 3844 spatial outputs per (n, co)
DSTARTS = [0, 8, 16, 24, 32, 40, 48, 54]  # d-block starts (last overlaps)
HCH = [(0, 8), (8, 8), (16, 8), (24, 8), (32, 8), (40, 8), (48, 8), (56, 6)]

LAST_EXEC_NS = None

_nc_cache = None


def _build_nc():
    f32 = mybir.dt.float32
    bf16 = mybir.dt.bfloat16
    nc = bacc.Bacc(None, target_bir_lowering=False)
    x = nc.dram_tensor("x", [NS, CIN, D, H, W], f32, kind="ExternalInput")
    lw = nc.dram_tensor("lw", [90, 3, 128], bf16, kind="ExternalInput")
    sel = nc.dram_tensor("sel", [16, 16], bf16, kind="ExternalInput")
    bia = nc.dram_tensor("bia", [16, 1], f32, kind="ExternalInput")
    y = nc.dram_tensor("y", [NS, CO, HO, WO], f32, kind="ExternalOutput")

    CDHW = CIN * D * H * W
    DHW = D * H * W
    HW = H * W

    with tile.TileContext(nc) as tc:
        with (
            tc.tile_pool(name="wpool", bufs=1) as wpool,
            tc.tile_pool(name="bpool", bufs=5) as bpool,
            tc.tile_pool(name="apool", bufs=2) as apool,
            tc.tile_pool(name="work", bufs=2) as work,
            tc.tile_pool(name="cpsum", bufs=4, space="PSUM") as cpsum,
            tc.tile_pool(name="spsum", bufs=2, space="PSUM") as spsum,
        ):
            L = wpool.tile([90, 3, 128], bf16)
            nc.sync.dma_start(out=L[:, :, :], in_=lw[:, :, :])
            SEL = wpool.tile([16, 16], bf16)
            nc.sync.dma_start(out=SEL[:, :], in_=sel[:, :])
            BIA = wpool.tile([16, 1], f32)
            nc.sync.dma_start(out=BIA[:, :], in_=bia[:, :])

            qeng = [nc.sync, nc.scalar]
            for n in range(NS):
                # bf16 acc[(dp,co), h', w'] with 2 pad rows
                acc = apool.tile([128, 64, WO], bf16, tag="acc")
                for bi, d0 in enumerate(DSTARTS):
                    # rhs tile: partition (kw, dr, ci); free = padded flat plane
                    bt = bpool.tile([90, 65, 64], bf16, tag="b")
                    btf = bt.rearrange("p a b -> p (a b)")
                    # kw=1 slice straight from HBM with inline f32->bf16 cast
                    nc.gpsimd.dma_start(
                        out=btf[30:60, 1 : 1 + HW],
                        in_=bass.AP(
                            x, n * CDHW + d0 * HW, [[HW, 10], [DHW, CIN], [1, HW]]
                        ),
                    )
                    # kw=0/2 copies of the same planes, shifted placement
                    qeng[bi % 2].dma_start(
                        out=btf[0:30, 2 : 2 + HW], in_=btf[30:60, 1 : 1 + HW]
                    )
                    qeng[(bi + 1) % 2].dma_start(
                        out=btf[60:90, 0:HW], in_=btf[30:60, 1 : 1 + HW]
                    )
                    for c, (h0, hs) in enumerate(HCH):
                        ps = cpsum.tile([128, 8, WO], f32, tag="cp")
                        for kh in range(3):
                            nc.tensor.matmul(
                                ps[:, :hs, :],
                                L[:, kh, :],
                                bt[:, h0 + kh : h0 + kh + hs, 2:64],
                                start=(kh == 0),
                                stop=(kh == 2),
                            )
                        if bi == 0:
                            nc.scalar.copy(
                                out=acc[:, h0 : h0 + hs, :], in_=ps[:, :hs, :]
                            )
                        elif (bi + c) % 2 == 0:
                            # direct DVE min against PSUM
                            nc.vector.tensor_tensor(
                                out=acc[:, h0 : h0 + hs, :],
                                in0=acc[:, h0 : h0 + hs, :],
                                in1=ps[:, :hs, :],
                                op=mybir.AluOpType.min,
                            )
                        else:
                            # ACT copies PSUM->bf16, DVE mins in fast 2x/4x mode
                            sc = work.tile([128, 8, WO], bf16, tag="sc", bufs=3)
                            nc.scalar.copy(out=sc[:, :hs, :], in_=ps[:, :hs, :])
                            nc.vector.tensor_tensor(
                                out=acc[:, h0 : h0 + hs, :],
                                in0=acc[:, h0 : h0 + hs, :],
                                in1=sc[:, :hs, :],
                                op=mybir.AluOpType.min,
                            )
                # tree-min over the 8 dp partition groups: DMA copies the
                # upper half down to base partition 0, DVE does the min.
                ts = work.tile([64, 64, WO], bf16, tag="ts")
                for half in (64, 32, 16):
                    nc.gpsimd.dma_start(
                        out=ts[0:half, :, :], in_=acc[half : 2 * half, :, :]
                    )
                    nc.vector.tensor_tensor(
                        out=acc[0:half, :, :],
                        in0=acc[0:half, :, :],
                        in1=ts[0:half, :, :],
                        op=mybir.AluOpType.min,
                    )
                # softmax over co on partitions 0..15
                e = work.tile([16, 64, WO], bf16, tag="e")
                nc.scalar.activation(
                    out=e[:, :, :],
                    in_=acc[0:16, :, :],
                    func=mybir.ActivationFunctionType.Exp,
                    bias=BIA[:, 0:1],
                )
                o = work.tile([16, 64, WO], f32, tag="o")
                for c, (h0, hs) in enumerate(HCH):
                    ss = spsum.tile([16, 8, WO], f32, tag="ss")
                    nc.tensor.matmul(
                        ss[:, :hs, :],
                        SEL[:, :],
                        e[:, h0 : h0 + hs, :],
                        start=True,
                        stop=True,
                    )
                    r = work.tile([16, 8, WO], f32, tag="r")
                    nc.vector.reciprocal_approx_fast(
                        out=r[:, :hs, :], in_=ss[:, :hs, :]
                    )
                    nc.vector.tensor_mul(
                        o[:, h0 : h0 + hs, :], e[:, h0 : h0 + hs, :], r[:, :hs, :]
                    )
                of = o.rearrange("p a b -> p (a b)")
                nc.sync.dma_start(
                    out=bass.AP(y, n * CO * SP, [[SP, 16], [1, SP]]),
                    in_=of[:, 0:SP],
                )
    nc.finalize()
    return nc


def _host_consts(weight, bias):
    # L[(kw,dr,ci), kh, (dp,co)] = w[co,ci,dr-dp,kh,kw] banded
    lw = np.zeros((90, 3, 128), np.float32)
    for kw in range(3):
        for dr in range(10):
            for ci in range(CIN):
                for kh in range(3):
                    for dp in range(8):
                        kd = dr - dp
                        if 0 <= kd < 3:
                            lw[kw * 30 + dr * 3 + ci, kh, dp * 16 : dp * 16 + 16] = (
                                weight[:, ci, kd, kh, kw]
                            )
    lw = lw.astype(ml_dtypes.bfloat16)
    sel = np.ones((16, 16), ml_dtypes.bfloat16)
    bia = bias.astype(np.float32).reshape(16, 1)
    return lw, sel, bia


def kernel(x, weight, bias, _trace=False):
    global LAST_EXEC_NS, _nc_cache
    x = np.ascontiguousarray(x, dtype=np.float32)
    lw, sel, bia = _host_consts(
        np.asarray(weight, np.float32), np.asarray(bias, np.float32)
    )
    if _nc_cache is None:
        _nc_cache = _build_nc()
    n_cores = 8
    in_maps = [
        {"x": np.ascontiguousarray(x[2 * k : 2 * k + 2]), "lw": lw, "sel": sel, "bia": bia}
        for k in range(n_cores)
    ]
    res = run_bass_kernel_spmd(_nc_cache, in_maps, list(range(n_cores)), trace=_trace)
    LAST_EXEC_NS = res.exec_time_ns
    out = np.concatenate([res.results[k]["y"] for k in range(n_cores)], axis=0)
    return out.astype(np.float32)


if __name__ == "__main__":
    rng = np.random.default_rng(0)
    x = rng.standard_normal((16, 3, 64, 64, 64), dtype=np.float32)
    w = rng.standard_normal((16, 3, 3, 3, 3), dtype=np.float32) / 9.0
    b = (rng.standard_normal(16) * 0.01).astype(np.float32)
    out = kernel(x, w, b)
    print("out", out.shape, out.dtype, out[0, :, 0, 0])


# revision 12
# speedup vs baseline: 8.3933x; 1.0084x over previous
"""Trainium2 Bass kernel: Conv3d(3->16, k=3, valid) + bias, min over D, softmax over C.

Full inputs: x [16,3,64,64,64], weight [16,3,3,3,3], bias [16].
Output: [16,16,62,62] f32.

Sharding: data-parallel, 2 samples per core across 8 cores.

Per-core algorithm v5 (d-banded bf16 matmul, direct cast loads):
  - Band over D: out partitions m = (dp of 8, co of 16) = 128, contraction
    q = (kw of 3, dr of 10, ci of 3) = 90, kh via 3 PSUM accumulation steps
    (free-dim row offset h'+kh). bf16 matmuls: 1 cycle/row, FWL weight loads.
  - rhs tile per (n, d-block): partition (kw,dr,ci) holds the bf16 (h,w)
    plane x[n,ci,d0+dr,:,:] at flat offset (2-kw) (kw shift baked into
    placement). Loaded as: one gpsimd SWDGE cast-DMA (f32 HBM -> bf16, 16KB
    descriptors) into the kw=1 partitions, then two SBUF->SBUF bf16 copies
    into the kw=0/2 partitions on the sync/scalar HWDGE queues.
  - min over d' merges the 8 d-block PSUMs into bf16 acc[(dp,co), h', w'];
    chunks alternate between a direct DVE min (PSUM f32 operand) and an
    ACT copy->bf16 scratch + DVE 4x-mode bf16 min, balancing both engines.
  - dp-group tree-min: gpsimd-queue DMA copies the upper partition half to
    base 0, DVE bf16 min; 3 steps -> m at partitions 0..15.
  - softmax over co: ACT exp(+bias) -> bf16, all-ones 16x16 matmul for the
    channel sum per 496-col chunk, DVE reciprocal_approx_fast + multiply,
    one fat output DMA per sample.
"""

import sys

for _p in ("/opt/trn_rl_repo",):
    if _p not in sys.path:
        sys.path.insert(0, _p)

import ml_dtypes
import numpy as np

import concourse.bass as bass
import concourse.tile as tile
from concourse import bacc, mybir
from concourse.bass_utils import run_bass_kernel_spmd

# Drop redundant LDWEIGHTS after tile legalization: when consecutive PE
# matmuls reuse the same stationary, the PE array already holds the weights,
# so the repeated Ldweights (which carry no dependency edges -- the matmuls
# keep the graph) can simply be removed from the schedule.
_orig_tile_legalize = tile.tile_legalize


def _tile_legalize_dedup_ldw(obib, nc_):
    out = _orig_tile_legalize(obib, nc_)
    for bb, insts in out.items():
        kept = []
        last_sig = None
        for inst in insts:
            if inst.engine == mybir.EngineType.PE:
                if isinstance(inst, mybir.InstLdweights):
                    sig = str(inst.ins[0]) if inst.ins else None
                    if (
                        sig is not None
                        and sig == last_sig
                        and not inst.descendants
                        and not inst.nosync_dependency_names()
                    ):
                        continue  # redundant reload of the resident weights
                    last_sig = sig
                elif not isinstance(inst, mybir.InstMatmult):
                    last_sig = None
            kept.append(inst)
        if len(kept) != len(insts):
            insts[:] = kept
    return out


tile.tile_legalize = _tile_legalize_dedup_ldw

NS, CIN, CO = 2, 3, 16  # samples per core, in/out channels
D = H = W = 64
DO = HO = WO = 62
SP = HO * WO  # 3844 spatial outputs per (n, co)
DSTARTS = [0, 8, 16, 24, 32, 40, 48, 54]  # d-block starts (last overlaps)
HCH = [(0, 8), (8, 8), (16, 8), (24, 8), (32, 8), (40, 8), (48, 8), (56, 6)]

LAST_EXEC_NS = None

_nc_cache = None


def _build_nc():
    f32 = mybir.dt.float32
    bf16 = mybir.dt.bfloat16
    nc = bacc.Bacc(None, target_bir_lowering=False)
    x = nc.dram_tensor("x", [NS, CIN, D, H, W], f32, kind="ExternalInput")
    lw = nc.dram_tensor("lw", [90, 3, 128], bf16, kind="ExternalInput")
    sel = nc.dram_tensor("sel", [16, 16], bf16, kind="ExternalInput")
    bia = nc.dram_tensor("bia", [16, 1], f32, kind="ExternalInput")
    y = nc.dram_tensor("y", [NS, CO, HO, WO], f32, kind="ExternalOutput")

    CDHW = CIN * D * H * W
    DHW = D * H * W
    HW = H * W

    with tile.TileContext(nc) as tc:
        with (
            tc.tile_pool(name="wpool", bufs=1) as wpool,
            tc.tile_pool(name="bpool", bufs=5) as bpool,
            tc.tile_pool(name="apool", bufs=2) as apool,
            tc.tile_pool(name="work", bufs=2) as work,
            tc.tile_pool(name="cpsum", bufs=6, space="PSUM") as cpsum,
            tc.tile_pool(name="spsum", bufs=2, space="PSUM") as spsum,
        ):
            L = wpool.tile([90, 3, 128], bf16)
            nc.sync.dma_start(out=L[:, :, :], in_=lw[:, :, :])
            SEL = wpool.tile([16, 16], bf16)
            nc.sync.dma_start(out=SEL[:, :], in_=sel[:, :])
            BIA = wpool.tile([16, 1], f32)
            nc.sync.dma_start(out=BIA[:, :], in_=bia[:, :])

            qeng = [nc.sync, nc.scalar]
            for n in range(NS):
                # bf16 acc[(dp,co), h', w'] with 2 pad rows
                acc = apool.tile([128, 64, WO], bf16, tag="acc")
                for bi, d0 in enumerate(DSTARTS):
                    # rhs tile: partition (kw, dr, ci); free = padded flat plane
                    bt = bpool.tile([90, 65, 64], bf16, tag="b")
                    btf = bt.rearrange("p a b -> p (a b)")
                    # kw=1 slice straight from HBM with inline f32->bf16 cast
                    nc.gpsimd.dma_start(
                        out=btf[30:60, 1 : 1 + HW],
                        in_=bass.AP(
                            x, n * CDHW + d0 * HW, [[HW, 10], [DHW, CIN], [1, HW]]
                        ),
                    )
                    # kw=0/2 copies of the same planes, shifted placement
                    qeng[bi % 2].dma_start(
                        out=btf[0:30, 2 : 2 + HW], in_=btf[30:60, 1 : 1 + HW]
                    )
                    qeng[(bi + 1) % 2].dma_start(
                        out=btf[60:90, 0:HW], in_=btf[30:60, 1 : 1 + HW]
                    )
                    for g, chunks in enumerate((HCH[0:3], HCH[3:6], HCH[6:8])):
                        pss = [
                            cpsum.tile([128, 8, WO], f32, tag="cp", name=f"cp{g}_{i_}")
                            for i_ in range(len(chunks))
                        ]
                        for kh in range(3):
                            for (h0, hs), ps in zip(chunks, pss):
                                nc.tensor.matmul(
                                    ps[:, :hs, :],
                                    L[:, kh, :],
                                    bt[:, h0 + kh : h0 + kh + hs, 2:64],
                                    start=(kh == 0),
                                    stop=(kh == 2),
                                )
                        for ci_, ((h0, hs), ps) in enumerate(zip(chunks, pss)):
                          c = 3 * g + ci_
                          if bi == 0:
                            nc.scalar.copy(
                                out=acc[:, h0 : h0 + hs, :], in_=ps[:, :hs, :]
                            )
                          elif (bi + c) % 2 == 0:
                            # direct DVE min against PSUM
                            nc.vector.tensor_tensor(
                                out=acc[:, h0 : h0 + hs, :],
                                in0=acc[:, h0 : h0 + hs, :],
                                in1=ps[:, :hs, :],
                                op=mybir.AluOpType.min,
                            )
                          else:
                            # ACT copies PSUM->bf16, DVE mins in fast 2x/4x mode
                            sc = work.tile([128, 8, WO], bf16, tag="sc", bufs=3)
                            nc.scalar.copy(out=sc[:, :hs, :], in_=ps[:, :hs, :])
                            nc.vector.tensor_tensor(
                                out=acc[:, h0 : h0 + hs, :],
                                in0=acc[:, h0 : h0 + hs, :],
                                in1=sc[:, :hs, :],
                                op=mybir.AluOpType.min,
                            )
                # tree-min over the 8 dp partition groups: DMA copies the
                # upper half down to base partition 0, DVE does the min.
                ts = work.tile([64, 64, WO], bf16, tag="ts")
                for half in (64, 32, 16):
                    nc.gpsimd.dma_start(
                        out=ts[0:half, :, :], in_=acc[half : 2 * half, :, :]
                    )
                    nc.vector.tensor_tensor(
                        out=acc[0:half, :, :],
                        in0=acc[0:half, :, :],
                        in1=ts[0:half, :, :],
                        op=mybir.AluOpType.min,
                    )
                # softmax over co on partitions 0..15
                e = work.tile([16, 64, WO], bf16, tag="e")
                nc.scalar.activation(
                    out=e[:, :, :],
                    in_=acc[0:16, :, :],
                    func=mybir.ActivationFunctionType.Exp,
                    bias=BIA[:, 0:1],
                )
                o = work.tile([16, 64, WO], f32, tag="o")
                for c, (h0, hs) in enumerate(HCH):
                    ss = spsum.tile([16, 8, WO], f32, tag="ss")
                    nc.tensor.matmul(
                        ss[:, :hs, :],
                        SEL[:, :],
                        e[:, h0 : h0 + hs, :],
                        start=True,
                        stop=True,
                    )
                    r = work.tile([16, 8, WO], f32, tag="r")
                    nc.vector.reciprocal_approx_fast(
                        out=r[:, :hs, :], in_=ss[:, :hs, :]
                    )
                    nc.vector.tensor_mul(
                        o[:, h0 : h0 + hs, :], e[:, h0 : h0 + hs, :], r[:, :hs, :]
                    )
                of = o.rearrange("p a b -> p (a b)")
                nc.sync.dma_start(
                    out=bass.AP(y, n * CO * SP, [[SP, 16], [1, SP]]),
                    in_=of[:, 0:SP],
                )
    nc.finalize()
    return nc


def _host_consts(weight, bias):
    # L[(kw,dr,ci), kh, (dp,co)] = w[co,ci,dr-dp,kh,kw] banded
    lw = np.zeros((90, 3, 128), np.float32)
    for kw in range(3):
        for dr in range(10):
            for ci in range(CIN):
                for kh in range(3):
                    for dp in range(8):
                        kd = dr - dp
                        if 0 <= kd < 3:
                            lw[kw * 30 + dr * 3 + ci, kh, dp * 16 : dp * 16 + 16] = (
                                weight[:, ci, kd, kh, kw]
                            )
    lw = lw.astype(ml_dtypes.bfloat16)
    sel = np.ones((16, 16), ml_dtypes.bfloat16)
    bia = bias.astype(np.float32).reshape(16, 1)
    return lw, sel, bia


def kernel(x, weight, bias, _trace=False):
    global LAST_EXEC_NS, _nc_cache
    x = np.ascontiguousarray(x, dtype=np.float32)
    lw, sel, bia = _host_consts(
        np.asarray(weight, np.float32), np.asarray(bias, np.float32)
    )
    if _nc_cache is None:
        _nc_cache = _build_nc()
    n_cores = 8
    in_maps = [
        {"x": np.ascontiguousarray(x[2 * k : 2 * k + 2]), "lw": lw, "sel": sel, "bia": bia}
        for k in range(n_cores)
    ]
    res = run_bass_kernel_spmd(_nc_cache, in_maps, list(range(n_cores)), trace=_trace)
    LAST_EXEC_NS = res.exec_time_ns
    out = np.concatenate([res.results[k]["y"] for k in range(n_cores)], axis=0)
    return out.astype(np.float32)


if __name__ == "__main__":
    rng = np.random.default_rng(0)
    x = rng.standard_normal((16, 3, 64, 64, 64), dtype=np.float32)
    w = rng.standard_normal((16, 3, 3, 3, 3), dtype=np.float32) / 9.0
    b = (rng.standard_normal(16) * 0.01).astype(np.float32)
    out = kernel(x, w, b)
    print("out", out.shape, out.dtype, out[0, :, 0, 0])


# revision 13
# speedup vs baseline: 8.8891x; 1.0591x over previous
"""Trainium2 Bass kernel: Conv3d(3->16, k=3, valid) + bias, min over D, softmax over C.

Full inputs: x [16,3,64,64,64], weight [16,3,3,3,3], bias [16].
Output: [16,16,62,62] f32.

Sharding: data-parallel, 2 samples per core across 8 cores.

Per-core algorithm v5 (d-banded bf16 matmul, direct cast loads):
  - Band over D: out partitions m = (dp of 8, co of 16) = 128, contraction
    q = (kw of 3, dr of 10, ci of 3) = 90, kh via 3 PSUM accumulation steps
    (free-dim row offset h'+kh). bf16 matmuls: 1 cycle/row, FWL weight loads.
  - rhs tile per (n, d-block): partition (kw,dr,ci) holds the bf16 (h,w)
    plane x[n,ci,d0+dr,:,:] at flat offset (2-kw) (kw shift baked into
    placement). Loaded as: one gpsimd SWDGE cast-DMA (f32 HBM -> bf16, 16KB
    descriptors) into the kw=1 partitions, then two SBUF->SBUF bf16 copies
    into the kw=0/2 partitions on the sync/scalar HWDGE queues.
  - min over d' merges the 8 d-block PSUMs into bf16 acc[(dp,co), h', w'];
    chunks alternate between a direct DVE min (PSUM f32 operand) and an
    ACT copy->bf16 scratch + DVE 4x-mode bf16 min, balancing both engines.
  - dp-group tree-min: gpsimd-queue DMA copies the upper partition half to
    base 0, DVE bf16 min; 3 steps -> m at partitions 0..15.
  - softmax over co: ACT exp(+bias) -> bf16, all-ones 16x16 matmul for the
    channel sum per 496-col chunk, DVE reciprocal_approx_fast + multiply,
    one fat output DMA per sample.
"""

import sys

for _p in ("/opt/trn_rl_repo",):
    if _p not in sys.path:
        sys.path.insert(0, _p)

import ml_dtypes
import numpy as np

import concourse.bass as bass
import concourse.tile as tile
from concourse import bacc, mybir
from concourse.bass_utils import run_bass_kernel_spmd

# Drop redundant LDWEIGHTS after tile legalization: when consecutive PE
# matmuls reuse the same stationary, the PE array already holds the weights,
# so the repeated Ldweights (which carry no dependency edges -- the matmuls
# keep the graph) can simply be removed from the schedule.
_orig_tile_legalize = tile.tile_legalize


def _tile_legalize_dedup_ldw(obib, nc_):
    out = _orig_tile_legalize(obib, nc_)
    for bb, insts in out.items():
        kept = []
        last_sig = None
        for inst in insts:
            if inst.engine == mybir.EngineType.PE:
                if isinstance(inst, mybir.InstLdweights):
                    sig = str(inst.ins[0]) if inst.ins else None
                    if (
                        sig is not None
                        and sig == last_sig
                        and not inst.descendants
                        and not inst.nosync_dependency_names()
                    ):
                        continue  # redundant reload of the resident weights
                    last_sig = sig
                elif not isinstance(inst, mybir.InstMatmult):
                    last_sig = None
            kept.append(inst)
        if len(kept) != len(insts):
            insts[:] = kept
    return out


tile.tile_legalize = _tile_legalize_dedup_ldw

NS, CIN, CO = 2, 3, 16  # samples per core, in/out channels
D = H = W = 64
DO = HO = WO = 62
SP = HO * WO  # 3844 spatial outputs per (n, co)
DSTARTS = [0, 8, 16, 24, 32, 40, 48, 54]  # d-block starts (last overlaps)
HCH = [(0, 8), (8, 8), (16, 8), (24, 8), (32, 8), (40, 8), (48, 8), (56, 6)]

LAST_EXEC_NS = None

_nc_cache = None


def _build_nc():
    f32 = mybir.dt.float32
    bf16 = mybir.dt.bfloat16
    nc = bacc.Bacc(None, target_bir_lowering=False)
    x = nc.dram_tensor("x", [NS, CIN, D, H, W], f32, kind="ExternalInput")
    lw = nc.dram_tensor("lw", [90, 3, 128], bf16, kind="ExternalInput")
    sel = nc.dram_tensor("sel", [16, 16], bf16, kind="ExternalInput")
    bia = nc.dram_tensor("bia", [16, 1], f32, kind="ExternalInput")
    y = nc.dram_tensor("y", [NS, CO, HO, WO], f32, kind="ExternalOutput")

    CDHW = CIN * D * H * W
    DHW = D * H * W
    HW = H * W

    with tile.TileContext(nc) as tc:
        with (
            tc.tile_pool(name="wpool", bufs=1) as wpool,
            tc.tile_pool(name="bpool", bufs=5) as bpool,
            tc.tile_pool(name="apool", bufs=2) as apool,
            tc.tile_pool(name="work", bufs=2) as work,
            tc.tile_pool(name="cpsum", bufs=6, space="PSUM") as cpsum,
            tc.tile_pool(name="spsum", bufs=2, space="PSUM") as spsum,
        ):
            L = wpool.tile([90, 3, 128], bf16)
            nc.sync.dma_start(out=L[:, :, :], in_=lw[:, :, :])
            SEL = wpool.tile([16, 16], bf16)
            nc.sync.dma_start(out=SEL[:, :], in_=sel[:, :])
            BIA = wpool.tile([16, 1], f32)
            nc.sync.dma_start(out=BIA[:, :], in_=bia[:, :])

            qeng = [nc.sync, nc.scalar]
            for n in range(NS):
                # bf16 acc[(dp,co), h', w'] with 2 pad rows
                acc = apool.tile([128, 64, WO], bf16, tag="acc")
                for bi, d0 in enumerate(DSTARTS):
                    # rhs tile: partition (kw, dr, ci); free = padded flat plane
                    bt = bpool.tile([90, 65, 64], bf16, tag="b")
                    btf = bt.rearrange("p a b -> p (a b)")
                    if bi == 0:
                        # pipeline fill: 3 parallel cast-DMAs from HBM
                        for kw in range(3):
                            nc.gpsimd.dma_start(
                                out=btf[kw * 30 : kw * 30 + 30, (2 - kw) : (2 - kw) + HW],
                                in_=bass.AP(
                                    x,
                                    n * CDHW + d0 * HW,
                                    [[HW, 10], [DHW, CIN], [1, HW]],
                                ),
                            )
                    else:
                        # kw=1 slice straight from HBM with inline f32->bf16
                        # cast, then two shifted SBUF copies for kw=0/2
                        nc.gpsimd.dma_start(
                            out=btf[30:60, 1 : 1 + HW],
                            in_=bass.AP(
                                x, n * CDHW + d0 * HW, [[HW, 10], [DHW, CIN], [1, HW]]
                            ),
                        )
                        qeng[bi % 2].dma_start(
                            out=btf[0:30, 2 : 2 + HW], in_=btf[30:60, 1 : 1 + HW]
                        )
                        qeng[(bi + 1) % 2].dma_start(
                            out=btf[60:90, 0:HW], in_=btf[30:60, 1 : 1 + HW]
                        )
                    for g, chunks in enumerate((HCH[0:3], HCH[3:6], HCH[6:8])):
                        pss = [
                            cpsum.tile([128, 8, WO], f32, tag="cp", name=f"cp{g}_{i_}")
                            for i_ in range(len(chunks))
                        ]
                        for kh in range(3):
                            for (h0, hs), ps in zip(chunks, pss):
                                nc.tensor.matmul(
                                    ps[:, :hs, :],
                                    L[:, kh, :],
                                    bt[:, h0 + kh : h0 + kh + hs, 2:64],
                                    start=(kh == 0),
                                    stop=(kh == 2),
                                )
                        for ci_, ((h0, hs), ps) in enumerate(zip(chunks, pss)):
                          c = 3 * g + ci_
                          if bi == 0:
                            nc.scalar.copy(
                                out=acc[:, h0 : h0 + hs, :], in_=ps[:, :hs, :]
                            )
                          elif (bi + c) % 2 == 0:
                            # direct DVE min against PSUM
                            nc.vector.tensor_tensor(
                                out=acc[:, h0 : h0 + hs, :],
                                in0=acc[:, h0 : h0 + hs, :],
                                in1=ps[:, :hs, :],
                                op=mybir.AluOpType.min,
                            )
                          else:
                            # ACT copies PSUM->bf16, DVE mins in fast 2x/4x mode
                            sc = work.tile([128, 8, WO], bf16, tag="sc", bufs=3)
                            nc.scalar.copy(out=sc[:, :hs, :], in_=ps[:, :hs, :])
                            nc.vector.tensor_tensor(
                                out=acc[:, h0 : h0 + hs, :],
                                in0=acc[:, h0 : h0 + hs, :],
                                in1=sc[:, :hs, :],
                                op=mybir.AluOpType.min,
                            )
                # per-chunk tail: dp-group tree-min (DMA copy down to base 0
                # + DVE min), then exp / channel-sum / reciprocal / multiply /
                # output DMA -- all per h-chunk so the tail pipelines under
                # the remaining conv matmuls.
                for c, (h0, hs) in enumerate(HCH):
                    ts = work.tile([64, 8, WO], bf16, tag="ts", bufs=3)
                    for half in (64, 32, 16):
                        nc.gpsimd.dma_start(
                            out=ts[0:half, :hs, :],
                            in_=acc[half : 2 * half, h0 : h0 + hs, :],
                        )
                        nc.vector.tensor_tensor(
                            out=acc[0:half, h0 : h0 + hs, :],
                            in0=acc[0:half, h0 : h0 + hs, :],
                            in1=ts[0:half, :hs, :],
                            op=mybir.AluOpType.min,
                        )
                    e = work.tile([16, 8, WO], bf16, tag="e", bufs=3)
                    nc.scalar.activation(
                        out=e[:, :hs, :],
                        in_=acc[0:16, h0 : h0 + hs, :],
                        func=mybir.ActivationFunctionType.Exp,
                        bias=BIA[:, 0:1],
                    )
                    ss = spsum.tile([16, 8, WO], f32, tag="ss")
                    nc.tensor.matmul(
                        ss[:, :hs, :],
                        SEL[:, :],
                        e[:, :hs, :],
                        start=True,
                        stop=True,
                    )
                    r = work.tile([16, 8, WO], f32, tag="r", bufs=3)
                    nc.vector.reciprocal_approx_fast(
                        out=r[:, :hs, :], in_=ss[:, :hs, :]
                    )
                    o = work.tile([16, 8, WO], f32, tag="o", bufs=3)
                    nc.vector.tensor_mul(o[:, :hs, :], e[:, :hs, :], r[:, :hs, :])
                    nc.sync.dma_start(
                        out=bass.AP(
                            y, n * CO * SP + h0 * WO, [[SP, 16], [1, hs * WO]]
                        ),
                        in_=o.rearrange("p a b -> p (a b)")[:, 0 : hs * WO],
                    )
    nc.finalize()
    return nc


def _host_consts(weight, bias):
    # L[(kw,dr,ci), kh, (dp,co)] = w[co,ci,dr-dp,kh,kw] banded
    lw = np.zeros((90, 3, 128), np.float32)
    for kw in range(3):
        for dr in range(10):
            for ci in range(CIN):
                for kh in range(3):
                    for dp in range(8):
                        kd = dr - dp
                        if 0 <= kd < 3:
                            lw[kw * 30 + dr * 3 + ci, kh, dp * 16 : dp * 16 + 16] = (
                                weight[:, ci, kd, kh, kw]
                            )
    lw = lw.astype(ml_dtypes.bfloat16)
    sel = np.ones((16, 16), ml_dtypes.bfloat16)
    bia = bias.astype(np.float32).reshape(16, 1)
    return lw, sel, bia


def kernel(x, weight, bias, _trace=False):
    global LAST_EXEC_NS, _nc_cache
    x = np.ascontiguousarray(x, dtype=np.float32)
    lw, sel, bia = _host_consts(
        np.asarray(weight, np.float32), np.asarray(bias, np.float32)
    )
    if _nc_cache is None:
        _nc_cache = _build_nc()
    n_cores = 8
    in_maps = [
        {"x": np.ascontiguousarray(x[2 * k : 2 * k + 2]), "lw": lw, "sel": sel, "bia": bia}
        for k in range(n_cores)
    ]
    res = run_bass_kernel_spmd(_nc_cache, in_maps, list(range(n_cores)), trace=_trace)
    LAST_EXEC_NS = res.exec_time_ns
    out = np.concatenate([res.results[k]["y"] for k in range(n_cores)], axis=0)
    return out.astype(np.float32)


if __name__ == "__main__":
    rng = np.random.default_rng(0)
    x = rng.standard_normal((16, 3, 64, 64, 64), dtype=np.float32)
    w = rng.standard_normal((16, 3, 3, 3, 3), dtype=np.float32) / 9.0
    b = (rng.standard_normal(16) * 0.01).astype(np.float32)
    out = kernel(x, w, b)
    print("out", out.shape, out.dtype, out[0, :, 0, 0])
